# revision 2
# baseline (speedup 1.0000x reference)
"""Trainium2 Bass kernel for nn_DenseGRU (8-core data-parallel over batch).

Math notes (exact algebraic simplifications of the reference):
  - The attention softmax is over a singleton axis -> weights == 1.0
    exactly, so ctx_t = x[:, :, t, :].sum(axis=1) and W1/b1/W2/b2/V/bV
    never affect the output.
  - gi_t = W_ih @ [ctx_t, gt] + b_ih is h-independent -> precomputed per
    chunk in one GEMM pass before the scan steps that consume it.
  - h' = (1-z).n + z.h  ==  n*wt + ut  with  wt=(1-z)/32, ut=z*(h/32),
    keeping the state ALWAYS as h/32 (the /32 cancels the fp8 weight
    scale SC=32, so every gate preactivation in PSUM is UNSCALED and
    the final fc folds the 32 into Wfc host-side).

Performance structure (per core = one batch sample):
  - W_hh stored fp8 E3M4 scaled by 32; moving state hs2 = h/32 in f16,
    so gh PSUM = W_hh @ h exactly (no activation-side descale needed).
  - Per-step critical path (the serial GRU chain) is minimized:
      PE:  rz-phase kc-major (8 MMs per kc), n-phase col-major,
           then pipeline units (next chunk's reduce/GI) in the tail idle.
      ACT: sig_z, sig_r, then 4x per-column FUSED
           n_c = tanh(ghn_c * scale=r_c + bias=gin_c)  (scale/bias are
           per-partition APs; gin copied to SBUF per chunk).
      DVE: ut = z*hs2, wt = -z/32 + 1/32, then per-column
           hs2[t+1,c] = (n_c * wt_c) + ut_c  (scalar_tensor_tensor),
           so next-step kc0 matmuls start after the FIRST column lands.
  - DMA rides 3 parallel queues: x halves on gpsimd-SWDGE + sync-HWDGE,
    weights on the scalar-HWDGE queue, so the scan starts ~7us in
    instead of ~28us.
  - PSUM discipline: each pg bank gets exactly one start=True (first
    touch); all later matmuls accumulate via per-element pending bits.
"""

import sys
import types
import numpy as np
from contextlib import ExitStack

import ml_dtypes
import concourse.bass as bass
import concourse.tile as tile
from concourse import bacc, mybir
from concourse.bass_utils import run_bass_kernel_spmd
from concourse.alu_op_type import AluOpType

F32 = mybir.dt.float32
F16 = mybir.dt.float16
F8 = mybir.dt.float8e3
NP8 = ml_dtypes.float8_e3m4

B, HW, T, C = 8, 256, 32, 512
H, OUT = 512, 4
G3 = 3 * H          # 1536 gate rows
NJG = G3 // 128     # 12 j-groups
NKC = H // 128      # 4 c-chunks
SC = 32.0           # fp8 weight scale (power of 2); state is h/SC

CHS = [4, 8, 10, 10]            # GI chunk sizes (sum = T)
CH0 = [0, 4, 12, 22]            # chunk start timesteps
NCH = len(CHS)

Sig = mybir.ActivationFunctionType.Sigmoid
Tanh = mybir.ActivationFunctionType.Tanh


def _raw(r):
    return r.ins if isinstance(r, bass.BassInstruction) else r


def build_program():
    nc = bacc.Bacc("TRN2", target_bir_lowering=False, debug=False,
                   enable_asserts=False, num_devices=8)

    xb = nc.dram_tensor("xb", [HW, T, C], F16, kind="ExternalInput")
    wih = nc.dram_tensor("wih", [C + OUT + 1, G3], F16, kind="ExternalInput")
    whh = nc.dram_tensor("whh", [H, G3], F8, kind="ExternalInput")
    aug = nc.dram_tensor("aug", [OUT + 1, T], F16, kind="ExternalInput")
    bhn = nc.dram_tensor("bhn", [NKC, 128], F16, kind="ExternalInput")
    sel = nc.dram_tensor("sel", [NKC, NKC, T], F16, kind="ExternalInput")
    wfc = nc.dram_tensor("wfc", [128, NKC * (OUT + 1)], F32, kind="ExternalInput")
    bfc = nc.dram_tensor("bfc", [1, OUT + 1], F32, kind="ExternalInput")
    oh = nc.dram_tensor("oh", [128, T], F32, kind="ExternalInput")
    outp = nc.dram_tensor("out", [1, OUT + 1], F32, kind="ExternalOutput")

    with tile.TileContext(nc) as tc, ExitStack() as ctx:
        def chain(key, r):
            tc.chain_iter_dep(key, _raw(r))

        const = ctx.enter_context(tc.tile_pool(name="const", bufs=1))
        xpool = ctx.enter_context(tc.tile_pool(name="xp", bufs=T // 4))
        psp = ctx.enter_context(tc.tile_pool(name="psp", bufs=1, space="PSUM"))
        spool = ctx.enter_context(tc.tile_pool(name="spool", bufs=2, space="PSUM"))
        gtp = ctx.enter_context(tc.tile_pool(name="gtp", bufs=3))
        selp = ctx.enter_context(tc.tile_pool(name="selp", bufs=1))

        # ---- persistent SBUF state ----
        hs2 = const.tile([128, T + 1, NKC], F16, tag="hs2")   # h / SC
        nc.vector.memset(hs2[:, 0, :], 0.0)
        ones_sb = const.tile([128, 1], F16, tag="ones")
        nc.vector.memset(ones_sb[:], 1.0)
        warm = const.tile([1, 1], F32, tag="warm")
        nc.vector.memset(warm[:], 0.0)
        chain("act", nc.scalar.activation(warm[:], warm[:], Sig))  # table load

        # S[c, t] = sum_hw x[t, c, hw] (f16); gins = unscaled gi_n in SBUF
        S = const.tile([128, NKC, T], F16, tag="S")
        gins = const.tile([128, NKC, T], F32, tag="gins")
        selt = const.tile([128, NKC, T], F32, tag="selt")

        # PSUM gate tiles, one column per timestep, whole-scan lifetime:
        #   pg_rz[:, 0] = gi_r + gh_r,  pg_rz[:, 1] = gi_z + gh_z
        #   pg_n[:, 0]  = gh_n + b_hh_n,  pg_n[:, 1] = gi_n   (all unscaled)
        pg_rz = psp.tile([128, 2, NKC, T], F32, tag="pg_rz")
        pg_n = psp.tile([128, 2, NKC, T], F32, tag="pg_n")
        wps = psp.tile([1, 1], F32, tag="wps")
        psfc = psp.tile([1, OUT + 1], F32, tag="psfc")

        # ---- PE warmup: trivial matmuls during the DMA fill (p-state) ----
        for i in range(60):
            chain("pe", nc.tensor.matmul(wps[:], ones_sb[:, 0:1], ones_sb[:, 0:1],
                                         start=True, stop=True,
                                         skip_group_check=True))

        # ---- DMA: three parallel queues.
        #   gpsimd (SWDGE):  x hh=0 halves, chunk 0 first
        #   sync   (HWDGE):  small tensors, x hh=1 chunk 0, whh, x hh=1 rest
        #   scalar (HWDGE):  wih tiles (GI-critical)
        TCH = 4                 # timesteps per x DMA chunk
        xtiles = {}

        def load_x_half(xc, hh):
            if xc not in xtiles:
                xtiles[xc] = xpool.tile([128, 2, TCH, C], F16, tag="x",
                                        name=f"x{xc}")
            eng = nc.gpsimd if hh == 0 else nc.sync
            key = "dmag" if hh == 0 else "dmas"
            chain(key, eng.dma_start(
                out=xtiles[xc][:, hh, :, :],
                in_=xb.ap()[hh * 128:(hh + 1) * 128,
                            xc * TCH:(xc + 1) * TCH, :]))

        # sync queue: smalls first
        aug_sb = const.tile([OUT + 1, T], F16, tag="aug")
        chain("dmas", nc.sync.dma_start(out=aug_sb[:], in_=aug.ap()))
        bhn_sb = const.tile([NKC, 128], F16, tag="bhn")
        chain("dmas", nc.sync.dma_start(out=bhn_sb[:], in_=bhn.ap()))
        sel_sb = const.tile([NKC, NKC, T], F16, tag="sel")
        chain("dmas", nc.sync.dma_start(out=sel_sb[:], in_=sel.ap()))
        oh_sb = const.tile([128, T], F32, tag="oh")
        chain("dmas", nc.sync.dma_start(out=oh_sb[:], in_=oh.ap()))
        wfc_sb = const.tile([128, NKC * (OUT + 1)], F32, tag="wfc")
        chain("dmas", nc.sync.dma_start(out=wfc_sb[:], in_=wfc.ap()))
        bfc_sb = const.tile([1, OUT + 1], F32, tag="bfc")
        chain("dmas", nc.sync.dma_start(out=bfc_sb[:], in_=bfc.ap()))

        # scalar queue: wih (GI-critical path)
        wih_sb = [const.tile([128, G3], F16, tag=f"wih{kc}", name=f"wih{kc}")
                  for kc in range(NKC)]
        for kc in range(NKC):
            chain("dmaa", nc.scalar.dma_start(
                out=wih_sb[kc][:], in_=wih.ap()[kc * 128:(kc + 1) * 128, :]))
        wih4_sb = const.tile([OUT + 1, G3], F16, tag="wih4")
        chain("dmaa", nc.scalar.dma_start(out=wih4_sb[:],
                                          in_=wih.ap()[C:C + OUT + 1, :]))

        # x chunk 0 on both halves, then whh on sync, then the rest of x
        load_x_half(0, 0)
        load_x_half(0, 1)
        whh_sb = [const.tile([128, G3], F8, tag=f"whh{kc}", name=f"whh{kc}")
                  for kc in range(NKC)]
        for kc in range(NKC):
            chain("dmas", nc.sync.dma_start(
                out=whh_sb[kc][:], in_=whh.ap()[kc * 128:(kc + 1) * 128, :]))
        for xc in range(1, T // TCH):
            load_x_half(xc, 0)
        for xc in range(1, T // TCH):
            load_x_half(xc, 1)

        # ---- building blocks ----
        # Reduce over HW on the tensor engine: stationary = x tile
        # [128 hw, 128 c], moving = ones -> psum column S_ps[:, kc, ti].
        sps = {}

        def reduce_t(cn, ti, hhs=(0, 1)):
            t = CH0[cn] + ti
            if cn not in sps:
                sps[cn] = spool.tile([128, NKC, 16], F32, tag="sps",
                                     name=f"sps{cn % 2}")
            xt = xtiles[t // TCH]
            tr = t % TCH
            for kc in range(NKC):
                cs = slice(kc * 128, (kc + 1) * 128)
                for hh in hhs:
                    chain("pe", nc.tensor.matmul(
                        sps[cn][:, kc, ti:ti + 1], xt[:, hh, tr, cs],
                        ones_sb[:],
                        start=(ti == 0 and kc == 0 and hh == 0 and 0 in hhs),
                        stop=(kc == 3 and hh == max(hhs)),
                        skip_group_check=True))

        def s_copy(cn):
            t0, sz = CH0[cn], CHS[cn]
            with nc.allow_low_precision("S fits f16; summands are O(16)"):
                chain("act", nc.scalar.copy(S[:, :, t0:t0 + sz],
                                            sps.pop(cn)[:, :, 0:sz]))

        def gin_copy(cn):
            t0, sz = CH0[cn], CHS[cn]
            chain("act", nc.scalar.copy(gins[:, :, t0:t0 + sz],
                                        pg_n[:, 1, :, t0:t0 + sz]))

        # PSUM discipline: start=True clears has_written for the WHOLE 2KB
        # bank -> exactly ONE start=True per pg bank (first touch).
        def bhn_preload(cn):
            t0, sz = CH0[cn], CHS[cn]
            for jg4 in range(NKC):
                chain("pe", nc.tensor.matmul(
                    pg_n[:, 0, jg4, t0:t0 + sz], bhn_sb[:],
                    sel_sb[:, jg4, t0:t0 + sz],
                    start=(cn == 0 and jg4 == 0), stop=False,
                    skip_group_check=True))

        def gi_jg(cn, jg):
            """GI for one j-group of a chunk, accumulated into scan PSUM."""
            t0, sz = CH0[cn], CHS[cn]
            g, jg4 = divmod(jg, NKC)
            dst = (pg_rz[:, g, jg4, t0:t0 + sz] if g < 2
                   else pg_n[:, 1, jg4, t0:t0 + sz])
            js = slice(jg * 128, (jg + 1) * 128)
            for kc in range(NKC):
                chain("pe", nc.tensor.matmul(
                    dst, wih_sb[kc][:, js], S[:, kc, t0:t0 + sz],
                    start=(cn == 0 and jg == 0 and kc == 0),
                    stop=False, skip_group_check=True))
            chain("pe", nc.tensor.matmul(
                dst, wih4_sb[:, js], aug_sb[:, t0:t0 + sz],
                start=False, stop=(g == 2), skip_group_check=True))

        # ---- the scan step ----
        # PE: rz-phase kc-major so next step's MMs consume hs2 columns in
        # the order the h' STTs produce them; n-phase col-major so each
        # tanh_c can fire as its column completes.
        def scan_step(t):
            mov = [hs2[:, t, kc:kc + 1] for kc in range(NKC)]
            for kc in range(NKC):
                for g in (0, 1):
                    for jg4 in range(NKC):
                        js = slice((g * NKC + jg4) * 128,
                                   (g * NKC + jg4 + 1) * 128)
                        chain("pe", nc.tensor.matmul(
                            pg_rz[:, g, jg4, t:t + 1], whh_sb[kc][:, js],
                            mov[kc], start=False, stop=(kc == 3),
                            skip_group_check=True))
            for jg4 in range(NKC):
                js = slice((2 * NKC + jg4) * 128, (2 * NKC + jg4 + 1) * 128)
                for kc in range(NKC):
                    chain("pe", nc.tensor.matmul(
                        pg_n[:, 0, jg4, t:t + 1], whh_sb[kc][:, js],
                        mov[kc], start=False, stop=(kc == 3),
                        skip_group_check=True))

            # ACT: z first (feeds early DVE products), r, then fused tanhs
            z_t = gtp.tile([128, NKC], F32, tag="z")
            chain("act", nc.scalar.activation(z_t[:], pg_rz[:, 1, :, t], Sig))
            r_t = gtp.tile([128, NKC], F32, tag="r")
            chain("act", nc.scalar.activation(r_t[:], pg_rz[:, 0, :, t], Sig))
            n_t = gtp.tile([128, NKC], F32, tag="n")
            for c in range(NKC):
                chain("act", nc.scalar.activation(
                    n_t[:, c:c + 1], pg_n[:, 0, c, t:t + 1], Tanh,
                    scale=r_t[:, c:c + 1], bias=gins[:, c, t:t + 1]))

            # DVE: ut = z * hs2(t), wt = (1-z)/SC, then per-column
            # hs2[t+1, c] = n_c*wt_c + ut_c  (pipelines into next step's MMs)
            u_t = gtp.tile([128, NKC], F32, tag="u")
            chain("dve", nc.vector.tensor_mul(u_t[:], z_t[:], hs2[:, t, :]))
            w_t = gtp.tile([128, NKC], F32, tag="w")
            chain("dve", nc.vector.tensor_scalar(
                w_t[:], z_t[:], -1.0 / SC, 1.0 / SC,
                AluOpType.mult, AluOpType.add))
            for c in range(NKC):
                chain("dve", nc.vector.scalar_tensor_tensor(
                    hs2[:, t + 1, c:c + 1], n_t[:, c:c + 1], w_t[:, c:c + 1],
                    u_t[:, c:c + 1], AluOpType.mult, AluOpType.add))
            # one-hot select accumulates per step, hidden in the tail idle
            chain("dve", nc.vector.tensor_scalar_mul(
                selt[:, :, t], hs2[:, t + 1, :], oh_sb[:, t:t + 1]))

        # ---- prologue: chunk 0 before the scan ----
        # hh=0 reduces first (its DMA lands first), then hh=1.
        for hh in (0, 1):
            for ti in range(CHS[0]):
                reduce_t(0, ti, hhs=(hh,))
        s_copy(0)
        bhn_preload(0)
        # kc-major so the first GI matmuls start as soon as wih_sb[0] lands
        t0, sz = CH0[0], CHS[0]
        for kc in range(NKC):
            for jg in range(NJG):
                g, jg4 = divmod(jg, NKC)
                dst = (pg_rz[:, g, jg4, t0:t0 + sz] if g < 2
                       else pg_n[:, 1, jg4, t0:t0 + sz])
                js = slice(jg * 128, (jg + 1) * 128)
                chain("pe", nc.tensor.matmul(
                    dst, wih_sb[kc][:, js], S[:, kc, t0:t0 + sz],
                    start=(jg == 0 and kc == 0),
                    stop=False, skip_group_check=True))
        for jg in range(NJG):
            g, jg4 = divmod(jg, NKC)
            dst = (pg_rz[:, g, jg4, t0:t0 + sz] if g < 2
                   else pg_n[:, 1, jg4, t0:t0 + sz])
            js = slice(jg * 128, (jg + 1) * 128)
            chain("pe", nc.tensor.matmul(
                dst, wih4_sb[:, js], aug_sb[:, t0:t0 + sz],
                start=False, stop=(g == 2), skip_group_check=True))
        gin_copy(0)

        # ---- scan with spread-in pipeline work (runs in the PE tail idle
        # after each step's n-phase) ----
        for cn in range(NCH):
            sz = CHS[cn]
            units = [[] for _ in range(sz)]
            if cn + 1 < NCH:
                reds = [("red", ti) for ti in range(CHS[cn + 1])]
                gis = ([("scopy", cn + 1)] + [("gi", jg) for jg in range(NJG)]
                       + [("gincopy", cn + 1)] + [("bhn", cn + 1)])
                nred_steps = max(1, (5 * sz) // 10)
                ngi_steps = max(1, sz - nred_steps)
                for i, u in enumerate(reds):
                    units[(i * nred_steps) // len(reds)].append(u)
                for i, u in enumerate(gis):
                    units[min(sz - 1,
                              nred_steps + (i * ngi_steps) // len(gis))].append(u)
            for ti in range(sz):
                scan_step(CH0[cn] + ti)
                for kind, arg in units[ti]:
                    if kind == "red":
                        reduce_t(cn + 1, arg)
                    elif kind == "gi":
                        gi_jg(cn + 1, arg)
                    elif kind == "scopy":
                        s_copy(arg)
                    elif kind == "gincopy":
                        gin_copy(arg)
                    else:
                        bhn_preload(arg)

        global _DBG
        _DBG = {"S": S, "hs2": hs2, "pg_rz": pg_rz, "pg_n": pg_n,
                "gins": gins}

        # ---- select h at t = ori_len-1 via one-hot, then final fc ----
        hsel = selp.tile([128, NKC], F32, tag="hsel")
        nc.vector.tensor_reduce(hsel[:], selt[:], axis=mybir.AxisListType.X,
                                op=mybir.AluOpType.add)
        for kc in range(NKC):
            chain("pe", nc.tensor.matmul(
                psfc[:], hsel[:, kc:kc + 1],
                wfc_sb[:, kc * (OUT + 1):(kc + 1) * (OUT + 1)],
                start=(kc == 0), stop=(kc == 3),
                skip_group_check=True))
        out_sb = selp.tile([1, OUT + 1], F32, tag="osb")
        nc.vector.tensor_add(out_sb[:], psfc[:], bfc_sb[:])
        nc.sync.dma_start(out=outp.ap(), in_=out_sb[:])

    nc.compile()
    return nc


_NC_CACHE = None


def _get_nc():
    global _NC_CACHE
    if _NC_CACHE is None:
        _NC_CACHE = build_program()
    return _NC_CACHE


def _install_profile_hook():
    """Make run_bass_kernel_spmd(trace=True) work in this container."""
    if "antenv.axon_hooks" in sys.modules:
        return
    try:
        import trn_agent_boot.trn_boot as _boot
        hook = _boot._ntff_profile_via_ctypes("/opt/axon/libaxon_pjrt.so")
    except Exception:
        hook = None
    shim = types.ModuleType("antenv.axon_hooks")
    shim.get_axon_ntff_profile_hook = lambda: hook
    shim.set_axon_ntff_profile_hook = lambda h: None
    sys.modules["antenv.axon_hooks"] = shim


def make_in_maps(x, ori_len, gt, W_ih, W_hh, b_ih, b_hh, Wfc, bfc):
    x = np.asarray(x, np.float32)
    ori_len = np.asarray(ori_len)
    gt = np.asarray(gt, np.float32)
    W_ih = np.asarray(W_ih, np.float32)
    W_hh = np.asarray(W_hh, np.float32)
    b_ih = np.asarray(b_ih, np.float32)
    b_hh = np.asarray(b_hh, np.float32)
    Wfc = np.asarray(Wfc, np.float32)
    bfc = np.asarray(bfc, np.float32)

    # gi path UNSCALED; fp8 gh is (32*W) @ (h/32) so also unscaled; the
    # state scale h/32 is undone by Wfc*32 at the end.
    wih_host = np.zeros((C + OUT + 1, G3), np.float32)
    wih_host[:C + OUT] = W_ih.T
    brz = b_ih + np.concatenate([b_hh[:2 * H], np.zeros(H, np.float32)])
    wih_host[C + OUT] = brz
    whh_host = np.ascontiguousarray(W_hh.T * SC).astype(NP8)
    bhn_host = b_hh[2 * H:].reshape(NKC, 128).astype(np.float16)
    sel_host = np.zeros((NKC, NKC, T), np.float16)
    for k in range(NKC):
        sel_host[k, k, :] = 1.0
    wfc_host = np.ascontiguousarray(
        (Wfc * SC).reshape(NKC, 128, OUT + 1).transpose(1, 0, 2).reshape(128, -1))
    bfc_host = bfc[None, :]
    idx = np.clip(ori_len.astype(np.int64) - 1, 0, T - 1)

    in_maps = []
    for b in range(B):
        aug_host = np.zeros((OUT + 1, T), np.float32)
        aug_host[:OUT] = gt[b][:, None]
        aug_host[OUT] = 1.0
        oh_host = np.zeros((T,), np.float32)
        oh_host[idx[b]] = 1.0
        in_maps.append({
            "xb": np.ascontiguousarray(x[b]).astype(np.float16),
            "wih": wih_host.astype(np.float16),
            "whh": whh_host,
            "aug": aug_host.astype(np.float16),
            "bhn": bhn_host,
            "sel": sel_host,
            "wfc": wfc_host,
            "bfc": bfc_host,
            "oh": np.repeat(oh_host[None, :], 128, axis=0),
        })
    return in_maps


def kernel(x, ori_len, gt, W1, b1, W2, b2, V, bV, W_ih, W_hh, b_ih, b_hh,
           Wfc, bfc, _trace=False):
    # Attention softmax is over a singleton dim -> weights == 1 exactly, so
    # ctx = x.sum(axis=1) and W1/b1/W2/b2/V/bV cannot affect the output.
    nc = _get_nc()
    in_maps = make_in_maps(x, ori_len, gt, W_ih, W_hh, b_ih, b_hh, Wfc, bfc)
    if _trace:
        _install_profile_hook()
    res = run_bass_kernel_spmd(nc, in_maps, list(range(B)), trace=_trace)
    out = np.stack([res.results[b]["out"][0] for b in range(B)]).astype(np.float32)
    if _trace:
        return out, res
    return out
